# revision 23
# baseline (speedup 1.0000x reference)
"""GNN linear-attention kernel for Trainium2 (8 NeuronCores, Bass/Tile).

Sharding: data-parallel over batch B=8 -- one graph (N=2048 nodes) per
NeuronCore; parameters replicated. The host ships two sharded arrays per
call: a small blob (x^T and weights as bf16, biases f32) and the adjacency
matrix bitpacked 8:1 (one contiguous packbits over the whole batch). The
Bass kernel unpacks the adjacency on-device, computes node degrees there,
runs the gate/QK/masked-attention/aggregate/normalize pipeline per core,
and returns bf16 outputs (one d2h transfer), cast to f32 on host. The
small-blob upload overlaps the host-side packbits of the adjacency.
"""
from contextlib import ExitStack
import math

import numpy as np
import ml_dtypes

B, N, D, O = 8, 2048, 128, 128
P = 128
NPBF16 = ml_dtypes.bfloat16

_cache = {}


# ---------------------------------------------------------------- blob layout
# blob per core: xw bf16 (D, N+D+2O) ++ pk u8 (N, N/8) ++ auxc f32 (D,3)
#                ++ auxr f32 (1, O)
def _blob_layout(n=N, d=D, o=O):
    j = n // 8
    xw_elems = d * (n + d + 2 * o)
    off_xw = 0
    off_pk = off_xw + xw_elems * 2
    off_auxc = off_pk + n * j
    off_auxr = off_auxc + d * 3 * 4
    size = off_auxr + o * 4
    return dict(J=j, off_xw=off_xw, off_pk=off_pk, off_auxc=off_auxc,
                off_auxr=off_auxr, size=size)


def _pack_xwa(x_b, W_qk, b_qk, W_l, b_l, W_r, W_d, b_d, out):
    n, d, o = N, D, O
    lay = _blob_layout()
    xw = out[lay["off_xw"]:lay["off_pk"]].view(NPBF16).reshape(d, n + d + 2 * o)
    xw[:, 0:n] = x_b.T
    xw[:, n:n + d] = W_qk
    xw[:, n + d:n + d + o] = W_l
    xw[:, n + d + o:] = W_r
    auxc = out[lay["off_auxc"]:lay["off_auxr"]].view(np.float32).reshape(d, 3)
    auxc[:, 0] = W_d[0]
    auxc[:, 1] = b_d
    auxc[:, 2] = b_qk
    auxr = out[lay["off_auxr"]:].view(np.float32)
    auxr[:] = b_l


# ---------------------------------------------------------------- bass kernel
def _build_nc():
    import concourse.tile as tile
    from concourse import bacc, mybir, masks

    F32 = mybir.dt.float32
    BF16 = mybir.dt.bfloat16
    U8 = mybir.dt.uint8

    lay = _blob_layout()
    J = lay["J"]
    T = N // P
    EPS_RS = 1e-6 * math.sqrt(D)

    nc = bacc.Bacc("TRN2", target_bir_lowering=False, debug=False)
    blob = nc.declare_dram_parameter("blob", [1, lay["size"]], U8, isOutput=False)
    out_d = nc.declare_dram_parameter("out", [N, O], BF16, isOutput=True)

    xa = blob.ap()
    xw_v = xa[:, lay["off_xw"]:lay["off_pk"]].bitcast(BF16) \
        .rearrange("1 (p f) -> p f", p=D)
    auxc_v = xa[:, lay["off_auxc"]:lay["off_auxr"]].bitcast(F32) \
        .rearrange("1 (p f) -> p f", p=D)
    auxr_v = xa[:, lay["off_auxr"]:lay["size"]].bitcast(F32)          # (1, O)
    pk_v = xa[:, lay["off_pk"]:lay["off_auxc"]] \
        .rearrange("1 (t p j) -> p t j", p=P, j=J)                    # (P,T,J)

    W = N + D + 2 * O

    with tile.TileContext(nc) as tc, ExitStack() as ctx:
        cpool = ctx.enter_context(tc.tile_pool(name="const", bufs=1))
        upool = ctx.enter_context(tc.tile_pool(name="unpack", bufs=2))
        wpool = ctx.enter_context(tc.tile_pool(name="work", bufs=3))
        spool = ctx.enter_context(tc.tile_pool(name="small", bufs=3))
        ps_s = ctx.enter_context(tc.tile_pool(name="ps_s", bufs=2, space="PSUM"))
        ps_tr = ctx.enter_context(tc.tile_pool(name="ps_tr", bufs=2, space="PSUM"))
        ps_agg = ctx.enter_context(tc.tile_pool(name="ps_agg", bufs=2, space="PSUM"))
        ps_big = ctx.enter_context(tc.tile_pool(name="ps_big", bufs=2, space="PSUM"))

        xw = cpool.tile([D, W], BF16)
        nc.sync.dma_start(xw[:], xw_v)
        auxc = cpool.tile([D, 3], F32)
        nc.sync.dma_start(auxc[:], auxc_v)
        auxr_sb = cpool.tile([1, O], F32)
        nc.sync.dma_start(auxr_sb[:], auxr_v)
        blr_bf = cpool.tile([1, O], BF16)
        nc.vector.tensor_copy(blr_bf[:], auxr_sb[:])
        pk = cpool.tile([P, T, J], U8)
        nc.sync.dma_start(pk[:], pk_v)
        ones_bf = cpool.tile([1, P], BF16)
        nc.vector.memset(ones_bf[:], 1.0)
        ident = cpool.tile([P, P], BF16)
        masks.make_identity(nc, ident[:])

        xt = xw[:, 0:N]
        wqk = xw[:, N:N + D]
        wl = xw[:, N + D:N + D + O]
        wr = xw[:, N + D + O:W]

        # ---- unpack adjacency to bf16 (n on partitions), degrees on the fly
        A_bf = cpool.tile([P, T, N], BF16)
        deg_cols = cpool.tile([P, T], F32)
        for nt in range(T):
            scr = upool.tile([P, N], U8, tag="scr")
            for g in range(8):
                nc.vector.tensor_scalar(
                    out=scr[:, g::8], in0=pk[:, nt, :],
                    scalar1=g, scalar2=1,
                    op0=mybir.AluOpType.logical_shift_right,
                    op1=mybir.AluOpType.bitwise_and)
            nc.vector.tensor_copy(A_bf[:, nt, :], scr[:])
            nc.vector.tensor_reduce(out=deg_cols[:, nt:nt + 1], in_=A_bf[:, nt, :],
                                    axis=mybir.AxisListType.X,
                                    op=mybir.AluOpType.add)
        # deg as rows: (P, T) f32 -> bf16 (exact: integer degrees) -> (T, P)
        deg_cols_bf = cpool.tile([P, T], BF16)
        nc.vector.tensor_copy(deg_cols_bf[:], deg_cols[:])
        ps_dg = ps_tr.tile([T, P], BF16, tag="tr")
        nc.tensor.transpose(ps_dg[:], deg_cols_bf[:], ident[:])
        deg_rows = cpool.tile([T, P], BF16)
        nc.vector.tensor_copy(deg_rows[:], ps_dg[:])
        deg_row = cpool.tile([1, N], BF16)
        nc.sync.dma_start(deg_row[:].rearrange("o (t p) -> o t p", t=T),
                          deg_rows[:])

        # ---- gate/xg in transposed (D, N) layout; deg broadcast across
        # partitions via a K=1 matmul with a ones column
        gateT = cpool.tile([D, N], BF16)
        GC = 512
        for c in range(N // GC):
            psg = ps_big.tile([P, GC], F32, tag="big")
            nc.tensor.matmul(psg[:], ones_bf[:], deg_row[:, c * GC:(c + 1) * GC],
                             start=True, stop=True)
            nc.scalar.activation(gateT[:, c * GC:(c + 1) * GC], psg[:],
                                 mybir.ActivationFunctionType.Sigmoid,
                                 bias=auxc[:, 1:2], scale=auxc[:, 0:1])
        xgT = cpool.tile([D, N], BF16)
        nc.vector.tensor_tensor(out=xgT[:], in0=xt, in1=gateT[:],
                                op=mybir.AluOpType.mult)

        # ---- QK^T = sigmoid(W_qk^T @ xgT + b_qk) : (D, N)
        QKT = cpool.tile([D, N], BF16)
        QC = 512
        for c in range(N // QC):
            psq = ps_big.tile([P, QC], F32, tag="big")
            nc.tensor.matmul(psq[:], wqk, xgT[:, c * QC:(c + 1) * QC],
                             start=True, stop=True)
            nc.scalar.activation(QKT[:, c * QC:(c + 1) * QC], psq[:],
                                 mybir.ActivationFunctionType.Sigmoid,
                                 bias=auxc[:, 2:3])

        # ---- xg natural layout (m on partitions) via PE transpose
        xgN = cpool.tile([P, T, D], BF16)
        for mt in range(T):
            pst = ps_tr.tile([P, P], BF16, tag="tr")
            nc.tensor.transpose(pst[:], xgT[:, mt * P:(mt + 1) * P], ident[:])
            nc.vector.tensor_copy(xgN[:, mt, :], pst[:])

        # ---- main loop over output row blocks
        for nb in range(T):
            psa = ps_agg.tile([P, D], F32, tag="agg")
            rs_parts = spool.tile([P, T], F32, tag="rsp")
            n0 = nb * P
            for mc in range(T):
                pss = ps_s.tile([P, P], F32, tag="s")
                nc.tensor.matmul(pss[:], QKT[:, n0:n0 + P],
                                 QKT[:, mc * P:(mc + 1) * P],
                                 start=True, stop=True)
                masked = wpool.tile([P, P], BF16, tag="masked")
                nc.vector.tensor_tensor(out=masked[:], in0=pss[:],
                                        in1=A_bf[:, nb, mc * P:(mc + 1) * P],
                                        op=mybir.AluOpType.mult)
                nc.vector.tensor_reduce(out=rs_parts[:, mc:mc + 1], in_=masked[:],
                                        axis=mybir.AxisListType.X,
                                        op=mybir.AluOpType.add)
                pst = ps_tr.tile([P, P], BF16, tag="tr")
                nc.tensor.transpose(pst[:], masked[:], ident[:])
                maskedT = wpool.tile([P, P], BF16, tag="maskedT")
                nc.vector.tensor_copy(maskedT[:], pst[:])
                nc.tensor.matmul(psa[:], maskedT[:], xgN[:, mc, :],
                                 start=(mc == 0), stop=(mc == T - 1))

            rs = spool.tile([P, 1], F32, tag="rs")
            nc.vector.tensor_reduce(out=rs[:], in_=rs_parts[:],
                                    axis=mybir.AxisListType.X,
                                    op=mybir.AluOpType.add)
            rcp = spool.tile([P, 1], F32, tag="rcp")
            nc.vector.tensor_scalar_add(rs[:], rs[:], EPS_RS)
            nc.vector.reciprocal(rcp[:], rs[:])
            agg_sb = spool.tile([P, D], BF16, tag="aggsb")
            nc.vector.tensor_scalar(out=agg_sb[:], in0=psa[:], scalar1=rcp[:],
                                    scalar2=None, op0=mybir.AluOpType.mult)
            pst2 = ps_tr.tile([P, P], BF16, tag="tr")
            nc.tensor.transpose(pst2[:], agg_sb[:], ident[:])
            aggT = spool.tile([P, D], BF16, tag="aggT")
            nc.vector.tensor_copy(aggT[:], pst2[:])

            pso = ps_big.tile([P, O], F32, tag="big")
            nc.tensor.matmul(pso[:], aggT[:], wl, start=True, stop=False)
            nc.tensor.matmul(pso[:], xgT[:, n0:n0 + P], wr, start=False, stop=False)
            nc.tensor.matmul(pso[:], ones_bf[:], blr_bf[:], start=False, stop=True)

            t = spool.tile([P, O], F32, tag="t")
            nc.vector.tensor_copy(t[:], pso[:])
            sq = spool.tile([P, O], F32, tag="sq")
            ss = spool.tile([P, 1], F32, tag="ss")
            nc.scalar.activation(sq[:], t[:], mybir.ActivationFunctionType.Square,
                                 accum_out=ss[:])
            ssi = spool.tile([P, 1], F32, tag="ssi")
            nc.vector.reciprocal(ssi[:], ss[:])
            rn = spool.tile([P, 1], F32, tag="rn")
            nc.scalar.activation(rn[:], ssi[:], mybir.ActivationFunctionType.Sqrt)
            nc.vector.tensor_scalar_min(rn[:], rn[:], 1e12)
            outb = spool.tile([P, O], BF16, tag="outb")
            nc.vector.tensor_scalar(out=outb[:], in0=t[:], scalar1=rn[:],
                                    scalar2=None, op0=mybir.AluOpType.mult)
            nc.sync.dma_start(out_d[n0:n0 + P, :], outb[:])

    nc.finalize()
    return nc


# ---------------------------------------------------------------- jax runner
def _get_rt():
    if "rt" in _cache:
        return _cache["rt"]
    import jax
    import jax.numpy as jnp
    from jax.experimental.shard_map import shard_map
    from jax.sharding import Mesh, PartitionSpec, NamedSharding
    from concourse import bass2jax, mybir

    nc = _build_nc()
    bass2jax.install_neuronx_cc_hook()

    partition_name = (nc.partition_id_tensor.name
                      if nc.partition_id_tensor else None)
    in_names, out_names, out_avals = [], [], []
    for alloc in nc.m.functions[0].allocations:
        if not isinstance(alloc, mybir.MemoryLocationSet):
            continue
        name = alloc.memorylocations[0].name
        if alloc.kind == "ExternalInput":
            if name != partition_name:
                in_names.append(name)
        elif alloc.kind == "ExternalOutput":
            out_names.append(name)
            out_avals.append(jax.core.ShapedArray(
                tuple(alloc.tensor_shape), mybir.dt.np(alloc.dtype)))
    assert in_names == ["blob"] and out_names == ["out"], (in_names, out_names)
    bind_names = in_names + out_names
    if partition_name is not None:
        bind_names = bind_names + [partition_name]

    def _body(*args):
        operands = list(args)
        if partition_name is not None:
            operands.append(bass2jax.partition_id_tensor())
        outs = bass2jax._bass_exec_p.bind(
            *operands,
            out_avals=tuple(out_avals),
            in_names=tuple(bind_names),
            out_names=tuple(out_names),
            lowering_input_output_aliases=(),
            sim_require_finite=True,
            sim_require_nnan=True,
            nc=nc,
        )
        return tuple(outs)

    devices = jax.devices()[:B]
    mesh = Mesh(np.asarray(devices), ("core",))
    spec = PartitionSpec("core")
    sharded = jax.jit(
        shard_map(_body, mesh=mesh, in_specs=(spec, spec),
                  out_specs=(spec,), check_rep=False),
        donate_argnums=(1,), keep_unused=True)
    zeros_fn = jax.jit(
        lambda: jnp.zeros((B * N, O), jnp.bfloat16),
        out_shardings=NamedSharding(mesh, spec))
    in_sharding = NamedSharding(mesh, spec)

    rt = dict(sharded=sharded, zeros_fn=zeros_fn, in_sharding=in_sharding,
              in_names=in_names, jax=jax)
    _cache["rt"] = rt
    return rt


def kernel(x, A, W_qk, b_qk, W_l, b_l, W_r, W_d, b_d):
    rt = _get_rt()
    args = (x, A, W_qk, b_qk, W_l, b_l, W_r, W_d, b_d)
    if not _cache.get("warmed"):
        # First call: run throwaway passes to warm the allocators, BLAS,
        # RPC/transfer paths and the donated-output cycle, so subsequent
        # calls run at steady state.
        _run_once(rt, *args)
        _run_once(rt, *args)
        _cache["warmed"] = True
    return _run_once(rt, *args)


def _run_once(rt, x, A, W_qk, b_qk, W_l, b_l, W_r, W_d, b_d):
    jax = rt["jax"]

    lay = _blob_layout()
    blob = _cache.get("blob_buf")
    if blob is None:
        blob = _cache["blob_buf"] = np.empty((B, lay["size"]), dtype=np.uint8)
    args = (W_qk, b_qk, W_l, b_l, W_r, W_d, b_d)
    for b in range(B):
        _pack_xwa(x[b], *args, out=blob[b])
    # adjacency bitpack: BLAS dot with bit weights beats np.packbits 2.4x
    # on this host; A is exactly 0.0/1.0 so the f32 byte values are exact
    w8 = (2.0 ** np.arange(8)).astype(np.float32)
    pk = A.reshape(-1, 8) @ w8
    blob[:, lay["off_pk"]:lay["off_auxc"]] = pk.reshape(B, -1)  # casts to u8

    dev_blob = jax.device_put(blob, rt["in_sharding"])
    zeros = rt["zeros_fn"]()  # device-side memset, donated to the NEFF output;
    # dispatched while the blob streams to the devices
    (out_g,) = rt["sharded"](dev_blob, zeros)
    res = np.asarray(out_g)
    return res.reshape(B, N, O).astype(np.float32)
